# revision 19
# baseline (speedup 1.0000x reference)
"""MoE layer (top-2 of 8 experts) on 8 TRN2 NeuronCores, expert-parallel.

Host side: router (exact replica of the reference jax ops, so top-k
selection bit-matches), token gather by expert assignment, weight
repacking into DMA-friendly bf16 layouts, and the final weighted
scatter-add.

Device side (one expert per core, SPMD): the full expert FFN
    h = X @ W1 ; act = gelu(h_gate) * h_up ; Y = act @ W2
in bf16 operands with fp32 PSUM accumulation (~4e-3 rel err), all
activations kept transposed (tokens on the free axis).

Schedule notes:
  * bf16 halves all DMA traffic and SBUF footprint vs fp32.
  * PE warm-up matmuls on a zeroed tile run during the initial DMA fill
    so the HAM clock gate releases to 8/8 (2.4 GHz) before real work.
  * ffn1 runs in j-blocks of 8 strips, two passes per block: pass A
    processes only the first token chunk (small xt footprint -> the
    first strip is compute-ready ~1us after the DMA rings open), pass B
    processes the remaining chunks on the same still-resident weight
    tiles. xt streams k-major, split across both HWDGE rings.
  * ko-outer matmul order reuses each stationary weight tile across all
    chunks of a pass (the bf16 LDWEIGHTS hides under the matmuls).
  * w2 streams during ffn2; output DMAs alternate rings; the final
    chunk is sub-split with its drain fanned across scalar+vector and
    both rings to shorten the kernel tail.

Self-contained: only library imports (numpy/jax/concourse), no file reads.
"""

import numpy as np

TOP_K = 2
EPS = 1e-6
P = 128
D = 2048
F = 2048  # expert hidden dim (ED)
E = 8
KO = D // P  # 16 K-tiles for matmul1 / output D-tiles
MJ = F // P  # 16 gate/up tile pairs; also K-tiles for matmul2
JB = 8  # ffn1 strip-block size (weights stay resident across both passes)

_BUILD_CACHE: dict = {}

# Activation for the gate branch. CoreSim doesn't implement Gelu, so tests
# can set this to "Identity" for structural sim validation.
ACT_FN = "Gelu"


def _chunks_of(C: int) -> list[tuple[int, int]]:
    """Split the token-capacity free axis into matmul chunks <= 512.

    512 fp32 PSUM values = exactly one 2 KiB bank, so each chunk's
    accumulator stays bank-aligned.
    """
    if C <= 512:
        return [(0, C)]
    nch = -(-C // 512)
    base = C // nch
    base -= base % 4
    sizes = [base] * nch
    rem = C - base * nch
    i = 0
    while rem > 0:
        add = min(4, rem)
        sizes[i % nch] += add
        rem -= add
        i += 1
    out = []
    off = 0
    for s in sizes:
        out.append((off, s))
        off += s
    assert off == C
    return out


def _build(C: int):
    """Build + compile the per-core expert-FFN bass program for capacity C."""
    key = (C, ACT_FN)
    if key in _BUILD_CACHE:
        return _BUILD_CACHE[key]

    import concourse.bacc as bacc
    import concourse.mybir as mybir
    import concourse.tile as tile
    f32 = mybir.dt.float32
    bf16 = mybir.dt.bfloat16
    act_fn = getattr(mybir.ActivationFunctionType, ACT_FN)
    ident = mybir.ActivationFunctionType.Identity
    chunks = _chunks_of(C)
    nch = len(chunks)
    assert nch <= 4

    nc = bacc.Bacc(
        "TRN2", target_bir_lowering=False, debug=False, enable_asserts=False
    )
    # Packed layouts (host pre-transposed, partition-major):
    #   xt[p, ko, c]    = X^T[ko*128+p, c]          (tokens on free axis)
    #   w1[p, m, ko, q] = W1perm[ko*128+p, m*128+q] (m: g0,u0,g1,u1,... strips)
    #   w2[p, i, fo, q] = W2[fo*128+p, i*128+q]
    #   yt[p, io, c]    = Y^T[io*128+p, c]
    c0_0, c0_n = chunks[0]
    # xt ships as two tensors so the chunk-0 head DMAs read contiguous
    # >=2KB per-partition lines (full DMA efficiency) before the rest lands.
    xta_d = nc.dram_tensor("xta", [P, KO, c0_n], bf16, kind="ExternalInput")
    xtb_d = (
        nc.dram_tensor("xtb", [P, KO, C - c0_n], bf16, kind="ExternalInput")
        if nch > 1
        else None
    )
    w1_d = nc.dram_tensor("w1", [P, 2 * MJ, KO, P], bf16, kind="ExternalInput")
    w2_d = nc.dram_tensor("w2", [P, KO, MJ, P], bf16, kind="ExternalInput")
    yt_d = nc.dram_tensor("yt", [P, KO, C], bf16, kind="ExternalOutput")

    with tile.TileContext(nc) as tc:
        with (
            tc.tile_pool(name="xt", bufs=1) as xt_pool,
            tc.tile_pool(name="act", bufs=1) as act_pool,
            tc.tile_pool(name="w1", bufs=2 * (JB + 2)) as w1_pool,
            tc.tile_pool(name="w2", bufs=5) as w2_pool,
            tc.tile_pool(name="tg", bufs=6) as tg_pool,
            tc.tile_pool(name="yo", bufs=6) as yo_pool,
            tc.tile_pool(name="wm", bufs=1) as wm_pool,
            tc.tile_pool(name="ps", bufs=8, space="PSUM") as ps_pool,
        ):
            # --- PE warm-up: dummy matmuls on a zeroed tile keep the HAM
            # clock-gate activity window busy while the first DMAs land.
            wtile = wm_pool.tile([P, P], bf16)
            nc.gpsimd.memset(wtile[:], 0.0)
            warm_ps = ps_pool.tile([P, 512], f32, tag="ps")
            for wi in range(48):
                mm = nc.tensor.matmul(
                    warm_ps[:, :P], wtile[:], wtile[:], start=True, stop=True
                )
                if wi:
                    mm.ins.ldweights = False

            xta_sb = xt_pool.tile([P, KO, c0_n], bf16)
            xtb_sb = (
                xt_pool.tile([P, KO, C - c0_n], bf16, name="xtb_sb")
                if nch > 1
                else None
            )
            act_sb = act_pool.tile([P, MJ, C], bf16)
            # ffn1 chunk descriptors: (xt tile, local col, global col, n)
            xchunks = [(xta_sb, 0, c0_0, c0_n)] + [
                (xtb_sb, c0 - c0_n, c0, cn) for c0, cn in chunks[1:]
            ]
            w1_tiles = {}

            def issue_w1(m, ring, halves=False):
                t = w1_pool.tile([P, KO, P], bf16, tag="w1s", name=f"w1_{m}")
                if halves:
                    ring.dma_start(t[:, : KO // 2], w1_d.ap()[:, m, : KO // 2])
                    ring.dma_start(t[:, KO // 2 :], w1_d.ap()[:, m, KO // 2 :])
                else:
                    ring.dma_start(t[:], w1_d.ap()[:, m])
                w1_tiles[m] = t

            # --- Head DMA schedule. xt chunk-0 k-quads split across both
            # rings, interleaved with half-tile weight strips for the first
            # three j's, so pass A's strip 0 streams without stalls.
            def xtc0_quad(ring, k0):
                ring.dma_start(
                    xta_sb[:, k0 : k0 + 4], xta_d.ap()[:, k0 : k0 + 4]
                )

            issue_w1(0, nc.scalar, halves=True)
            xtc0_quad(nc.sync, 0)
            xtc0_quad(nc.scalar, 4)
            xtc0_quad(nc.sync, 8)
            xtc0_quad(nc.scalar, 12)
            issue_w1(1, nc.sync, halves=True)
            issue_w1(2, nc.scalar, halves=True)
            issue_w1(3, nc.sync, halves=True)
            issue_w1(4, nc.scalar)
            issue_w1(5, nc.sync)
            # Remaining strip pairs for block 0, then the rest of xt (needed
            # only by pass B, ~40us later), so weight pairs stay ahead.
            for m in range(6, 2 * JB):
                issue_w1(m, nc.sync if m % 2 else nc.scalar)
            if nch > 1:
                # Scheduling floor: xtb is needed only by pass B (~45us in);
                # without this the scheduler interleaves these bulky loads
                # ahead of the pass-A weight strips and starves the PE.
                with tc.tile_wait_until(0.018):
                    for si, k0 in enumerate(range(0, KO, 2)):
                        ring = nc.sync if si % 2 else nc.scalar
                        ring.dma_start(
                            xtb_sb[:, k0 : k0 + 2], xtb_d.ap()[:, k0 : k0 + 2]
                        )

            def g_pass(wg, j, cs):
                pg = [
                    ps_pool.tile([P, 512], f32, tag="ps", name=f"pg{j}_{ci}")
                    for ci in range(len(cs))
                ]
                for ko in range(KO):
                    for ci, (xs, lc0, c0, cn) in enumerate(cs):
                        mm = nc.tensor.matmul(
                            pg[ci][:, :cn],
                            wg[:, ko],
                            xs[:, ko, lc0 : lc0 + cn],
                            start=(ko == 0),
                            stop=(ko == KO - 1),
                        )
                        if ci:  # same stationary as previous MM: skip reload
                            mm.ins.ldweights = False
                tg = []
                for ci, (xs, lc0, c0, cn) in enumerate(cs):
                    t = tg_pool.tile([P, 512], f32, tag="tg", name=f"tg{j}_{ci}")
                    nc.scalar.activation(t[:, :cn], pg[ci][:, :cn], act_fn)
                    tg.append(t)
                return tg

            def u_pass(wu, j, cs, tg):
                pu = [
                    ps_pool.tile([P, 512], f32, tag="ps", name=f"pu{j}_{ci}")
                    for ci in range(len(cs))
                ]
                for ko in range(KO):
                    for ci, (xs, lc0, c0, cn) in enumerate(cs):
                        mm = nc.tensor.matmul(
                            pu[ci][:, :cn],
                            wu[:, ko],
                            xs[:, ko, lc0 : lc0 + cn],
                            start=(ko == 0),
                            stop=(ko == KO - 1),
                        )
                        if ci:
                            mm.ins.ldweights = False
                for ci, (xs, lc0, c0, cn) in enumerate(cs):
                    nc.vector.tensor_mul(
                        out=act_sb[:, j, c0 : c0 + cn],
                        in0=tg[ci][:, :cn],
                        in1=pu[ci][:, :cn],
                    )

            with nc.named_scope("ffn1"):
                for jb in range(0, MJ, JB):
                    if jb > 0:  # prefetch this block's strip pairs
                        for m in range(2 * jb, 2 * (jb + JB)):
                            issue_w1(m, nc.sync if m % 2 else nc.scalar)
                    # pass A: first chunk only
                    for j in range(jb, jb + JB):
                        tg = g_pass(w1_tiles[2 * j], j, xchunks[:1])
                        u_pass(w1_tiles[2 * j + 1], j, xchunks[:1], tg)
                    # pass B: remaining chunks on the resident weight tiles
                    if nch > 1:
                        for j in range(jb, jb + JB):
                            wg = w1_tiles.pop(2 * j)
                            wu = w1_tiles.pop(2 * j + 1)
                            tg = g_pass(wg, j, xchunks[1:])
                            u_pass(wu, j, xchunks[1:], tg)
                    else:
                        for j in range(jb, jb + JB):
                            w1_tiles.pop(2 * j)
                            w1_tiles.pop(2 * j + 1)

            w2_tiles = {}

            def issue_w2(i, ring):
                t = w2_pool.tile([P, MJ, P], bf16, tag="w2s", name=f"w2_{i}")
                ring.dma_start(t[:], w2_d.ap()[:, i])
                w2_tiles[i] = t

            for i in range(3):
                issue_w2(i, nc.sync if i % 2 else nc.scalar)
            with nc.named_scope("ffn2"):
                for i in range(KO):
                    if i + 3 < KO:
                        issue_w2(i + 3, nc.sync if i % 2 else nc.scalar)
                    w2t = w2_tiles.pop(i)
                    last = i == KO - 1
                    cs = list(chunks)
                    if last and cs[-1][1] > 192:
                        # Sub-split the final chunk and run the last tile
                        # chunk-inner so each chunk's drain overlaps the next
                        # chunk's chains; the tail is one small copy + DMA.
                        lc0, lcn = cs.pop()
                        h = lcn - 96
                        cs += [(lc0, h), (lc0 + h, 96)]

                    def drain(ci, c0, cn):
                        yo = yo_pool.tile(
                            [P, 512], bf16, tag="yo", name=f"yo{i}_{ci}"
                        )
                        if last and ci == len(cs) - 1:
                            nc.scalar.activation(yo[:, :cn], py[ci][:, :cn], ident)
                            ring = nc.scalar
                        elif last and ci == len(cs) - 2:
                            nc.vector.tensor_copy(out=yo[:, :cn], in_=py[ci][:, :cn])
                            ring = nc.sync
                        else:
                            nc.vector.tensor_copy(out=yo[:, :cn], in_=py[ci][:, :cn])
                            ring = nc.sync if (i + ci) % 2 else nc.scalar
                        ring.dma_start(yt_d.ap()[:, i, c0 : c0 + cn], yo[:, :cn])

                    if last:
                        py = []
                        for ci, (c0, cn) in enumerate(cs):
                            py.append(ps_pool.tile(
                                [P, 512], f32, tag="ps", name=f"py{i}_{ci}"
                            ))
                            for fo in range(MJ):
                                nc.tensor.matmul(
                                    py[ci][:, :cn],
                                    w2t[:, fo],
                                    act_sb[:, fo, c0 : c0 + cn],
                                    start=(fo == 0),
                                    stop=(fo == MJ - 1),
                                )
                            drain(ci, c0, cn)
                    else:
                        py = [
                            ps_pool.tile(
                                [P, 512], f32, tag="ps", name=f"py{i}_{ci}"
                            )
                            for ci in range(len(cs))
                        ]
                        for fo in range(MJ):
                            for ci, (c0, cn) in enumerate(cs):
                                mm = nc.tensor.matmul(
                                    py[ci][:, :cn],
                                    w2t[:, fo],
                                    act_sb[:, fo, c0 : c0 + cn],
                                    start=(fo == 0),
                                    stop=(fo == MJ - 1),
                                )
                                if ci:
                                    mm.ins.ldweights = False
                        for ci, (c0, cn) in enumerate(cs):
                            drain(ci, c0, cn)

    nc.compile()
    _BUILD_CACHE[key] = nc
    return nc


def _router(x, router_scale, gate_w):
    """Replicate the reference router ops exactly (same jax ops, default
    backend) so the top-2 expert selection bit-matches the reference."""
    import jax
    import jax.numpy as jnp

    x = jnp.asarray(x)
    router_scale = jnp.asarray(router_scale)
    gate_w = jnp.asarray(gate_w)
    _B, _L, d = x.shape
    h = x * jax.lax.rsqrt(jnp.mean(x * x, axis=-1, keepdims=True) + EPS)
    h = h * (d**-0.5) * router_scale
    logits = (h @ gate_w).astype(jnp.float32)
    probs = jax.nn.softmax(logits, axis=-1)
    w, idx = jax.lax.top_k(probs, TOP_K)
    w = w / jnp.clip(jnp.sum(w, axis=-1, keepdims=True), 1e-12)
    w = w.astype(x.dtype)
    return (
        np.asarray(idx).reshape(-1, TOP_K),
        np.asarray(w).reshape(-1, TOP_K).astype(np.float32),
    )


def _bf16_dtype():
    import concourse.mybir as mybir

    return mybir.dt.np(mybir.dt.bfloat16)


def _pack_w1(gate_up_e: np.ndarray) -> np.ndarray:
    """[D, 2F] -> [P, 2*MJ, KO, P] bf16, gate/up 128-col strips interleaved."""
    g = gate_up_e[:, :F].reshape(D, MJ, P)
    u = gate_up_e[:, F:].reshape(D, MJ, P)
    w1p = np.empty((D, 2 * MJ, P), np.float32)
    w1p[:, 0::2] = g
    w1p[:, 1::2] = u
    # [D, 2MJ, P] -> [KO, P, 2MJ, P] -> [P, 2MJ, KO, P]
    return np.ascontiguousarray(
        w1p.reshape(KO, P, 2 * MJ, P).transpose(1, 2, 0, 3)
    ).astype(_bf16_dtype())


def _pack_w2(down_e: np.ndarray) -> np.ndarray:
    """[F, D] -> [P, KO, MJ, P] bf16 (w2[p, i, fo, q] = W2[fo*128+p, i*128+q])."""
    return np.ascontiguousarray(
        down_e.reshape(MJ, P, KO, P).transpose(1, 2, 0, 3)
    ).astype(_bf16_dtype())


def run_moe(x, router_scale, gate_w, gate_up, down, per_expert_scale, trace=False):
    from concourse import bass_utils

    x = np.asarray(x, dtype=np.float32)
    router_scale = np.asarray(router_scale, dtype=np.float32)
    gate_w = np.asarray(gate_w, dtype=np.float32)
    gate_up = np.asarray(gate_up, dtype=np.float32)
    down = np.asarray(down, dtype=np.float32)
    per_expert_scale = np.asarray(per_expert_scale, dtype=np.float32)

    B, L, d = x.shape
    N = B * L
    assert d == D and gate_up.shape == (E, D, 2 * F) and down.shape == (E, F, D)

    idxf, wf = _router(x, router_scale, gate_w)

    pair_expert = idxf.reshape(-1)
    pair_token = np.repeat(np.arange(N), TOP_K)
    pair_w = wf.reshape(-1) * per_expert_scale[pair_expert]

    order = np.argsort(pair_expert, kind="stable")
    tok_o = pair_token[order]
    w_o = pair_w[order]
    counts = np.bincount(pair_expert, minlength=E)
    offs = np.zeros(E + 1, np.int64)
    offs[1:] = np.cumsum(counts)

    # SBUF budget caps per-launch capacity; extreme routing imbalance falls
    # back to multiple launches over row segments of each expert's list.
    CMAX = 1296
    nseg = max(1, -(-int(counts.max()) // CMAX))
    seg_cap = -(-int(counts.max()) // nseg)
    C = max(64, -(-seg_cap // 4) * 4)

    nc = _build(C)
    c0_n = _chunks_of(C)[0][1]

    bf16 = _bf16_dtype()
    xf = x.reshape(N, D)
    w1_packed = [_pack_w1(gate_up[e]) for e in range(E)]
    w2_packed = [_pack_w2(down[e]) for e in range(E)]

    contrib = np.empty((len(tok_o), D), np.float32)
    res = None
    for s in range(nseg):
        in_maps = []
        ranges = []
        for e in range(E):
            lo = min(offs[e] + s * C, offs[e + 1])
            hi = min(lo + C, offs[e + 1])
            toks = tok_o[lo:hi]
            ranges.append((lo, hi))
            xg = np.zeros((C, D), np.float32)
            xg[: len(toks)] = xf[toks]
            xt = xg.T.reshape(KO, P, C).transpose(1, 0, 2).astype(bf16)
            im = {
                "xta": np.ascontiguousarray(xt[:, :, :c0_n]),
                "w1": w1_packed[e],
                "w2": w2_packed[e],
            }
            if C > c0_n:
                im["xtb"] = np.ascontiguousarray(xt[:, :, c0_n:])
            in_maps.append(im)

        res = bass_utils.run_bass_kernel_spmd(
            nc, in_maps, core_ids=list(range(E)), trace=trace and s == 0
        )
        for e in range(E):
            lo, hi = ranges[e]
            yt = np.asarray(res.results[e]["yt"]).astype(np.float32)
            ytd = yt.transpose(1, 0, 2).reshape(D, C)  # [D, C]
            contrib[lo:hi] = ytd[:, : hi - lo].T

    contrib *= w_o[:, None]

    s = np.argsort(tok_o, kind="stable")
    tok_s = tok_o[s]
    out = np.zeros((N, D), np.float32)
    if len(tok_s) == 2 * N and np.array_equal(tok_s[0::2], tok_s[1::2]):
        cs = contrib[s]
        out[tok_s[0::2]] = cs[0::2] + cs[1::2]
    else:  # defensive fallback (duplicate experts per token can't happen)
        np.add.at(out, tok_o, contrib)
    return out.reshape(B, L, D), res


def kernel(x, router_scale, gate_w, gate_up, down, per_expert_scale):
    out, _ = run_moe(x, router_scale, gate_w, gate_up, down, per_expert_scale)
    return out


# revision 20
# speedup vs baseline: 1.0064x; 1.0064x over previous
"""MoE layer (top-2 of 8 experts) on 8 TRN2 NeuronCores, expert-parallel.

Host side: router (exact replica of the reference jax ops, so top-k
selection bit-matches), token gather by expert assignment, weight
repacking into DMA-friendly bf16 layouts, and the final weighted
scatter-add.

Device side (one expert per core, SPMD): the full expert FFN
    h = X @ W1 ; act = gelu(h_gate) * h_up ; Y = act @ W2
in bf16 operands with fp32 PSUM accumulation (~4e-3 rel err), all
activations kept transposed (tokens on the free axis).

Schedule notes:
  * bf16 halves all DMA traffic and SBUF footprint vs fp32.
  * PE warm-up matmuls on a zeroed tile run during the initial DMA fill
    so the HAM clock gate releases to 8/8 (2.4 GHz) before real work.
  * ffn1 runs in j-blocks of 8 strips, two passes per block: pass A
    processes only the first token chunk (small xt footprint -> the
    first strip is compute-ready ~1us after the DMA rings open), pass B
    processes the remaining chunks on the same still-resident weight
    tiles. xt streams k-major, split across both HWDGE rings.
  * ko-outer matmul order reuses each stationary weight tile across all
    chunks of a pass (the bf16 LDWEIGHTS hides under the matmuls).
  * w2 streams during ffn2; output DMAs alternate rings; the final
    chunk is sub-split with its drain fanned across scalar+vector and
    both rings to shorten the kernel tail.

Self-contained: only library imports (numpy/jax/concourse), no file reads.
"""

import numpy as np

TOP_K = 2
EPS = 1e-6
P = 128
D = 2048
F = 2048  # expert hidden dim (ED)
E = 8
KO = D // P  # 16 K-tiles for matmul1 / output D-tiles
MJ = F // P  # 16 gate/up tile pairs; also K-tiles for matmul2
JB = 8  # ffn1 strip-block size (weights stay resident across both passes)

_BUILD_CACHE: dict = {}

# Activation for the gate branch. CoreSim doesn't implement Gelu, so tests
# can set this to "Identity" for structural sim validation.
ACT_FN = "Gelu"


def _chunks_of(C: int) -> list[tuple[int, int]]:
    """Split the token-capacity free axis into matmul chunks <= 512.

    512 fp32 PSUM values = exactly one 2 KiB bank, so each chunk's
    accumulator stays bank-aligned.
    """
    if C <= 512:
        return [(0, C)]
    nch = -(-C // 512)
    base = C // nch
    base -= base % 4
    sizes = [base] * nch
    rem = C - base * nch
    i = 0
    while rem > 0:
        add = min(4, rem)
        sizes[i % nch] += add
        rem -= add
        i += 1
    out = []
    off = 0
    for s in sizes:
        out.append((off, s))
        off += s
    assert off == C
    return out


def _build(C: int):
    """Build + compile the per-core expert-FFN bass program for capacity C."""
    key = (C, ACT_FN)
    if key in _BUILD_CACHE:
        return _BUILD_CACHE[key]

    import concourse.bacc as bacc
    import concourse.mybir as mybir
    import concourse.tile as tile
    f32 = mybir.dt.float32
    bf16 = mybir.dt.bfloat16
    act_fn = getattr(mybir.ActivationFunctionType, ACT_FN)
    ident = mybir.ActivationFunctionType.Identity
    chunks = _chunks_of(C)
    nch = len(chunks)
    assert nch <= 4

    nc = bacc.Bacc(
        "TRN2", target_bir_lowering=False, debug=False, enable_asserts=False
    )
    # Packed layouts (host pre-transposed, partition-major):
    #   xt[p, ko, c]    = X^T[ko*128+p, c]          (tokens on free axis)
    #   w1[p, m, ko, q] = W1perm[ko*128+p, m*128+q] (m: g0,u0,g1,u1,... strips)
    #   w2[p, i, fo, q] = W2[fo*128+p, i*128+q]
    #   yt[p, io, c]    = Y^T[io*128+p, c]
    c0_0, c0_n = chunks[0]
    # xt ships as two tensors so the chunk-0 head DMAs read contiguous
    # >=2KB per-partition lines (full DMA efficiency) before the rest lands.
    xta_d = nc.dram_tensor("xta", [P, KO, c0_n], bf16, kind="ExternalInput")
    xtb_d = (
        nc.dram_tensor("xtb", [P, KO, C - c0_n], bf16, kind="ExternalInput")
        if nch > 1
        else None
    )
    w1_d = nc.dram_tensor("w1", [P, 2 * MJ, KO, P], bf16, kind="ExternalInput")
    w2_d = nc.dram_tensor("w2", [P, KO, MJ, P], bf16, kind="ExternalInput")
    yt_d = nc.dram_tensor("yt", [P, KO, C], bf16, kind="ExternalOutput")

    with tile.TileContext(nc) as tc:
        with (
            tc.tile_pool(name="xt", bufs=1) as xt_pool,
            tc.tile_pool(name="act", bufs=1) as act_pool,
            tc.tile_pool(name="w1", bufs=2 * (JB + 2)) as w1_pool,
            tc.tile_pool(name="w2", bufs=5) as w2_pool,
            tc.tile_pool(name="tg", bufs=6) as tg_pool,
            tc.tile_pool(name="yo", bufs=6) as yo_pool,
            tc.tile_pool(name="wm", bufs=1) as wm_pool,
            tc.tile_pool(name="ps", bufs=8, space="PSUM") as ps_pool,
        ):
            # --- PE warm-up: dummy matmuls on a zeroed tile keep the HAM
            # clock-gate activity window busy while the first DMAs land.
            wtile = wm_pool.tile([P, P], bf16)
            nc.gpsimd.memset(wtile[:], 0.0)
            warm_ps = ps_pool.tile([P, 512], f32, tag="ps")
            for _ in range(48):
                nc.tensor.matmul(
                    warm_ps[:, :P], wtile[:], wtile[:], start=True, stop=True
                )

            xta_sb = xt_pool.tile([P, KO, c0_n], bf16)
            xtb_sb = (
                xt_pool.tile([P, KO, C - c0_n], bf16, name="xtb_sb")
                if nch > 1
                else None
            )
            act_sb = act_pool.tile([P, MJ, C], bf16)
            # ffn1 chunk descriptors: (xt tile, local col, global col, n)
            xchunks = [(xta_sb, 0, c0_0, c0_n)] + [
                (xtb_sb, c0 - c0_n, c0, cn) for c0, cn in chunks[1:]
            ]
            w1_tiles = {}

            def issue_w1(m, ring, halves=False):
                t = w1_pool.tile([P, KO, P], bf16, tag="w1s", name=f"w1_{m}")
                if halves:
                    ring.dma_start(t[:, : KO // 2], w1_d.ap()[:, m, : KO // 2])
                    ring.dma_start(t[:, KO // 2 :], w1_d.ap()[:, m, KO // 2 :])
                else:
                    ring.dma_start(t[:], w1_d.ap()[:, m])
                w1_tiles[m] = t

            # --- Head DMA schedule. xt chunk-0 k-quads split across both
            # rings, interleaved with half-tile weight strips for the first
            # three j's, so pass A's strip 0 streams without stalls.
            def xtc0_quad(ring, k0):
                ring.dma_start(
                    xta_sb[:, k0 : k0 + 4], xta_d.ap()[:, k0 : k0 + 4]
                )

            issue_w1(0, nc.scalar, halves=True)
            xtc0_quad(nc.sync, 0)
            xtc0_quad(nc.scalar, 4)
            xtc0_quad(nc.sync, 8)
            xtc0_quad(nc.scalar, 12)
            issue_w1(1, nc.sync, halves=True)
            issue_w1(2, nc.scalar, halves=True)
            issue_w1(3, nc.sync, halves=True)
            issue_w1(4, nc.scalar)
            issue_w1(5, nc.sync)
            # Remaining strip pairs for block 0, then the rest of xt (needed
            # only by pass B, ~40us later), so weight pairs stay ahead.
            for m in range(6, 2 * JB):
                issue_w1(m, nc.sync if m % 2 else nc.scalar)
            if nch > 1:
                # Scheduling floor: xtb is needed only by pass B (~45us in);
                # without this the scheduler interleaves these bulky loads
                # ahead of the pass-A weight strips and starves the PE.
                with tc.tile_wait_until(0.018):
                    for si, k0 in enumerate(range(0, KO, 2)):
                        ring = nc.sync if si % 2 else nc.scalar
                        ring.dma_start(
                            xtb_sb[:, k0 : k0 + 2], xtb_d.ap()[:, k0 : k0 + 2]
                        )

            def g_pass(wg, j, cs):
                pg = [
                    ps_pool.tile([P, 512], f32, tag="ps", name=f"pg{j}_{ci}")
                    for ci in range(len(cs))
                ]
                for ko in range(KO):
                    for ci, (xs, lc0, c0, cn) in enumerate(cs):
                        nc.tensor.matmul(
                            pg[ci][:, :cn],
                            wg[:, ko],
                            xs[:, ko, lc0 : lc0 + cn],
                            start=(ko == 0),
                            stop=(ko == KO - 1),
                        )
                tg = []
                for ci, (xs, lc0, c0, cn) in enumerate(cs):
                    t = tg_pool.tile([P, 512], f32, tag="tg", name=f"tg{j}_{ci}")
                    nc.scalar.activation(t[:, :cn], pg[ci][:, :cn], act_fn)
                    tg.append(t)
                return tg

            def u_pass(wu, j, cs, tg):
                pu = [
                    ps_pool.tile([P, 512], f32, tag="ps", name=f"pu{j}_{ci}")
                    for ci in range(len(cs))
                ]
                for ko in range(KO):
                    for ci, (xs, lc0, c0, cn) in enumerate(cs):
                        nc.tensor.matmul(
                            pu[ci][:, :cn],
                            wu[:, ko],
                            xs[:, ko, lc0 : lc0 + cn],
                            start=(ko == 0),
                            stop=(ko == KO - 1),
                        )
                for ci, (xs, lc0, c0, cn) in enumerate(cs):
                    nc.vector.tensor_mul(
                        out=act_sb[:, j, c0 : c0 + cn],
                        in0=tg[ci][:, :cn],
                        in1=pu[ci][:, :cn],
                    )

            with nc.named_scope("ffn1"):
                for jb in range(0, MJ, JB):
                    if jb > 0:  # prefetch this block's strip pairs
                        for m in range(2 * jb, 2 * (jb + JB)):
                            issue_w1(m, nc.sync if m % 2 else nc.scalar)
                    # pass A: first chunk only
                    for j in range(jb, jb + JB):
                        tg = g_pass(w1_tiles[2 * j], j, xchunks[:1])
                        u_pass(w1_tiles[2 * j + 1], j, xchunks[:1], tg)
                    # pass B: remaining chunks on the resident weight tiles
                    if nch > 1:
                        for j in range(jb, jb + JB):
                            wg = w1_tiles.pop(2 * j)
                            wu = w1_tiles.pop(2 * j + 1)
                            tg = g_pass(wg, j, xchunks[1:])
                            u_pass(wu, j, xchunks[1:], tg)
                    else:
                        for j in range(jb, jb + JB):
                            w1_tiles.pop(2 * j)
                            w1_tiles.pop(2 * j + 1)

            w2_tiles = {}

            def issue_w2(i, ring):
                t = w2_pool.tile([P, MJ, P], bf16, tag="w2s", name=f"w2_{i}")
                ring.dma_start(t[:], w2_d.ap()[:, i])
                w2_tiles[i] = t

            for i in range(3):
                issue_w2(i, nc.sync if i % 2 else nc.scalar)
            with nc.named_scope("ffn2"):
                for i in range(KO):
                    if i + 3 < KO:
                        issue_w2(i + 3, nc.sync if i % 2 else nc.scalar)
                    w2t = w2_tiles.pop(i)
                    last = i == KO - 1
                    cs = list(chunks)
                    if last and cs[-1][1] > 192:
                        # Sub-split the final chunk and run the last tile
                        # chunk-inner so each chunk's drain overlaps the next
                        # chunk's chains; the tail is one small copy + DMA.
                        lc0, lcn = cs.pop()
                        h = lcn - 96
                        cs += [(lc0, h), (lc0 + h, 96)]

                    def drain(ci, c0, cn):
                        yo = yo_pool.tile(
                            [P, 512], bf16, tag="yo", name=f"yo{i}_{ci}"
                        )
                        if last and ci == len(cs) - 1:
                            nc.scalar.activation(yo[:, :cn], py[ci][:, :cn], ident)
                            ring = nc.scalar
                        elif last and ci == len(cs) - 2:
                            nc.vector.tensor_copy(out=yo[:, :cn], in_=py[ci][:, :cn])
                            ring = nc.sync
                        else:
                            nc.vector.tensor_copy(out=yo[:, :cn], in_=py[ci][:, :cn])
                            ring = nc.sync if (i + ci) % 2 else nc.scalar
                        ring.dma_start(yt_d.ap()[:, i, c0 : c0 + cn], yo[:, :cn])

                    if last:
                        py = []
                        for ci, (c0, cn) in enumerate(cs):
                            py.append(ps_pool.tile(
                                [P, 512], f32, tag="ps", name=f"py{i}_{ci}"
                            ))
                            for fo in range(MJ):
                                nc.tensor.matmul(
                                    py[ci][:, :cn],
                                    w2t[:, fo],
                                    act_sb[:, fo, c0 : c0 + cn],
                                    start=(fo == 0),
                                    stop=(fo == MJ - 1),
                                )
                            drain(ci, c0, cn)
                    else:
                        py = [
                            ps_pool.tile(
                                [P, 512], f32, tag="ps", name=f"py{i}_{ci}"
                            )
                            for ci in range(len(cs))
                        ]
                        for fo in range(MJ):
                            for ci, (c0, cn) in enumerate(cs):
                                nc.tensor.matmul(
                                    py[ci][:, :cn],
                                    w2t[:, fo],
                                    act_sb[:, fo, c0 : c0 + cn],
                                    start=(fo == 0),
                                    stop=(fo == MJ - 1),
                                )
                        for ci, (c0, cn) in enumerate(cs):
                            drain(ci, c0, cn)

    nc.compile()
    _BUILD_CACHE[key] = nc
    return nc


def _router(x, router_scale, gate_w):
    """Replicate the reference router ops exactly (same jax ops, default
    backend) so the top-2 expert selection bit-matches the reference."""
    import jax
    import jax.numpy as jnp

    x = jnp.asarray(x)
    router_scale = jnp.asarray(router_scale)
    gate_w = jnp.asarray(gate_w)
    _B, _L, d = x.shape
    h = x * jax.lax.rsqrt(jnp.mean(x * x, axis=-1, keepdims=True) + EPS)
    h = h * (d**-0.5) * router_scale
    logits = (h @ gate_w).astype(jnp.float32)
    probs = jax.nn.softmax(logits, axis=-1)
    w, idx = jax.lax.top_k(probs, TOP_K)
    w = w / jnp.clip(jnp.sum(w, axis=-1, keepdims=True), 1e-12)
    w = w.astype(x.dtype)
    return (
        np.asarray(idx).reshape(-1, TOP_K),
        np.asarray(w).reshape(-1, TOP_K).astype(np.float32),
    )


def _bf16_dtype():
    import concourse.mybir as mybir

    return mybir.dt.np(mybir.dt.bfloat16)


def _pack_w1(gate_up_e: np.ndarray) -> np.ndarray:
    """[D, 2F] -> [P, 2*MJ, KO, P] bf16, gate/up 128-col strips interleaved."""
    g = gate_up_e[:, :F].reshape(D, MJ, P)
    u = gate_up_e[:, F:].reshape(D, MJ, P)
    w1p = np.empty((D, 2 * MJ, P), np.float32)
    w1p[:, 0::2] = g
    w1p[:, 1::2] = u
    # [D, 2MJ, P] -> [KO, P, 2MJ, P] -> [P, 2MJ, KO, P]
    return np.ascontiguousarray(
        w1p.reshape(KO, P, 2 * MJ, P).transpose(1, 2, 0, 3)
    ).astype(_bf16_dtype())


def _pack_w2(down_e: np.ndarray) -> np.ndarray:
    """[F, D] -> [P, KO, MJ, P] bf16 (w2[p, i, fo, q] = W2[fo*128+p, i*128+q])."""
    return np.ascontiguousarray(
        down_e.reshape(MJ, P, KO, P).transpose(1, 2, 0, 3)
    ).astype(_bf16_dtype())


def run_moe(x, router_scale, gate_w, gate_up, down, per_expert_scale, trace=False):
    from concourse import bass_utils

    x = np.asarray(x, dtype=np.float32)
    router_scale = np.asarray(router_scale, dtype=np.float32)
    gate_w = np.asarray(gate_w, dtype=np.float32)
    gate_up = np.asarray(gate_up, dtype=np.float32)
    down = np.asarray(down, dtype=np.float32)
    per_expert_scale = np.asarray(per_expert_scale, dtype=np.float32)

    B, L, d = x.shape
    N = B * L
    assert d == D and gate_up.shape == (E, D, 2 * F) and down.shape == (E, F, D)

    idxf, wf = _router(x, router_scale, gate_w)

    pair_expert = idxf.reshape(-1)
    pair_token = np.repeat(np.arange(N), TOP_K)
    pair_w = wf.reshape(-1) * per_expert_scale[pair_expert]

    order = np.argsort(pair_expert, kind="stable")
    tok_o = pair_token[order]
    w_o = pair_w[order]
    counts = np.bincount(pair_expert, minlength=E)
    offs = np.zeros(E + 1, np.int64)
    offs[1:] = np.cumsum(counts)

    # SBUF budget caps per-launch capacity; extreme routing imbalance falls
    # back to multiple launches over row segments of each expert's list.
    CMAX = 1296
    nseg = max(1, -(-int(counts.max()) // CMAX))
    seg_cap = -(-int(counts.max()) // nseg)
    C = max(64, -(-seg_cap // 4) * 4)

    nc = _build(C)
    c0_n = _chunks_of(C)[0][1]

    bf16 = _bf16_dtype()
    xf = x.reshape(N, D)
    w1_packed = [_pack_w1(gate_up[e]) for e in range(E)]
    w2_packed = [_pack_w2(down[e]) for e in range(E)]

    contrib = np.empty((len(tok_o), D), np.float32)
    res = None
    for s in range(nseg):
        in_maps = []
        ranges = []
        for e in range(E):
            lo = min(offs[e] + s * C, offs[e + 1])
            hi = min(lo + C, offs[e + 1])
            toks = tok_o[lo:hi]
            ranges.append((lo, hi))
            xg = np.zeros((C, D), np.float32)
            xg[: len(toks)] = xf[toks]
            xt = xg.T.reshape(KO, P, C).transpose(1, 0, 2).astype(bf16)
            im = {
                "xta": np.ascontiguousarray(xt[:, :, :c0_n]),
                "w1": w1_packed[e],
                "w2": w2_packed[e],
            }
            if C > c0_n:
                im["xtb"] = np.ascontiguousarray(xt[:, :, c0_n:])
            in_maps.append(im)

        res = bass_utils.run_bass_kernel_spmd(
            nc, in_maps, core_ids=list(range(E)), trace=trace and s == 0
        )
        for e in range(E):
            lo, hi = ranges[e]
            yt = np.asarray(res.results[e]["yt"]).astype(np.float32)
            ytd = yt.transpose(1, 0, 2).reshape(D, C)  # [D, C]
            contrib[lo:hi] = ytd[:, : hi - lo].T

    contrib *= w_o[:, None]

    s = np.argsort(tok_o, kind="stable")
    tok_s = tok_o[s]
    out = np.zeros((N, D), np.float32)
    if len(tok_s) == 2 * N and np.array_equal(tok_s[0::2], tok_s[1::2]):
        cs = contrib[s]
        out[tok_s[0::2]] = cs[0::2] + cs[1::2]
    else:  # defensive fallback (duplicate experts per token can't happen)
        np.add.at(out, tok_o, contrib)
    return out.reshape(B, L, D), res


def kernel(x, router_scale, gate_w, gate_up, down, per_expert_scale):
    out, _ = run_moe(x, router_scale, gate_w, gate_up, down, per_expert_scale)
    return out


# revision 21
# speedup vs baseline: 1.0123x; 1.0059x over previous
"""MoE layer (top-2 of 8 experts) on 8 TRN2 NeuronCores, expert-parallel.

Host side: router (exact replica of the reference jax ops, so top-k
selection bit-matches), token gather by expert assignment, weight
repacking into DMA-friendly bf16 layouts, and the final weighted
scatter-add.

Device side (one expert per core, SPMD): the full expert FFN
    h = X @ W1 ; act = gelu(h_gate) * h_up ; Y = act @ W2
in bf16 operands with fp32 PSUM accumulation (~4e-3 rel err), all
activations kept transposed (tokens on the free axis).

Schedule notes:
  * bf16 halves all DMA traffic and SBUF footprint vs fp32.
  * PE warm-up matmuls on a zeroed tile run during the initial DMA fill
    so the HAM clock gate releases to 8/8 (2.4 GHz) before real work.
  * ffn1 runs in j-blocks of 8 strips, two passes per block: pass A
    processes only the first token chunk (small xt footprint -> the
    first strip is compute-ready ~1us after the DMA rings open), pass B
    processes the remaining chunks on the same still-resident weight
    tiles. xt streams k-major, split across both HWDGE rings.
  * ko-outer matmul order reuses each stationary weight tile across all
    chunks of a pass (the bf16 LDWEIGHTS hides under the matmuls).
  * w2 streams during ffn2; output DMAs alternate rings; the final
    chunk is sub-split with its drain fanned across scalar+vector and
    both rings to shorten the kernel tail.

Self-contained: only library imports (numpy/jax/concourse), no file reads.
"""

import numpy as np

TOP_K = 2
EPS = 1e-6
P = 128
D = 2048
F = 2048  # expert hidden dim (ED)
E = 8
KO = D // P  # 16 K-tiles for matmul1 / output D-tiles
MJ = F // P  # 16 gate/up tile pairs; also K-tiles for matmul2
JB = 8  # ffn1 strip-block size (weights stay resident across both passes)

_BUILD_CACHE: dict = {}

# Activation for the gate branch. CoreSim doesn't implement Gelu, so tests
# can set this to "Identity" for structural sim validation.
ACT_FN = "Gelu"


def _chunks_of(C: int) -> list[tuple[int, int]]:
    """Split the token-capacity free axis into matmul chunks <= 512.

    512 fp32 PSUM values = exactly one 2 KiB bank, so each chunk's
    accumulator stays bank-aligned.
    """
    if C <= 512:
        return [(0, C)]
    nch = -(-C // 512)
    base = C // nch
    base -= base % 4
    sizes = [base] * nch
    rem = C - base * nch
    i = 0
    while rem > 0:
        add = min(4, rem)
        sizes[i % nch] += add
        rem -= add
        i += 1
    out = []
    off = 0
    for s in sizes:
        out.append((off, s))
        off += s
    assert off == C
    return out


def _build(C: int):
    """Build + compile the per-core expert-FFN bass program for capacity C."""
    key = (C, ACT_FN)
    if key in _BUILD_CACHE:
        return _BUILD_CACHE[key]

    import concourse.bacc as bacc
    import concourse.mybir as mybir
    import concourse.tile as tile
    f32 = mybir.dt.float32
    bf16 = mybir.dt.bfloat16
    act_fn = getattr(mybir.ActivationFunctionType, ACT_FN)
    ident = mybir.ActivationFunctionType.Identity
    chunks = _chunks_of(C)
    nch = len(chunks)
    assert nch <= 4

    nc = bacc.Bacc(
        "TRN2", target_bir_lowering=False, debug=False, enable_asserts=False
    )
    # Packed layouts (host pre-transposed, partition-major):
    #   xt[p, ko, c]    = X^T[ko*128+p, c]          (tokens on free axis)
    #   w1[p, m, ko, q] = W1perm[ko*128+p, m*128+q] (m: g0,u0,g1,u1,... strips)
    #   w2[p, i, fo, q] = W2[fo*128+p, i*128+q]
    #   yt[p, io, c]    = Y^T[io*128+p, c]
    c0_0, c0_n = chunks[0]
    # xt ships as two tensors so the chunk-0 head DMAs read contiguous
    # >=2KB per-partition lines (full DMA efficiency) before the rest lands.
    xta_d = nc.dram_tensor("xta", [P, KO, c0_n], bf16, kind="ExternalInput")
    xtb_d = (
        nc.dram_tensor("xtb", [P, KO, C - c0_n], bf16, kind="ExternalInput")
        if nch > 1
        else None
    )
    w1_d = nc.dram_tensor("w1", [P, 2 * MJ, KO, P], bf16, kind="ExternalInput")
    w2_d = nc.dram_tensor("w2", [P, KO, MJ, P], bf16, kind="ExternalInput")
    yt_d = nc.dram_tensor("yt", [P, KO, C], bf16, kind="ExternalOutput")

    with tile.TileContext(nc) as tc:
        with (
            tc.tile_pool(name="xt", bufs=1) as xt_pool,
            tc.tile_pool(name="act", bufs=1) as act_pool,
            tc.tile_pool(name="w1", bufs=2 * (JB + 2)) as w1_pool,
            tc.tile_pool(name="w2", bufs=5) as w2_pool,
            tc.tile_pool(name="tg", bufs=6) as tg_pool,
            tc.tile_pool(name="yo", bufs=6) as yo_pool,
            tc.tile_pool(name="wm", bufs=1) as wm_pool,
            tc.tile_pool(name="ps", bufs=8, space="PSUM") as ps_pool,
        ):
            # --- PE warm-up: dummy matmuls on a zeroed tile keep the HAM
            # clock-gate activity window busy while the first DMAs land.
            wtile = wm_pool.tile([P, P], bf16)
            nc.gpsimd.memset(wtile[:], 0.0)
            warm_ps = ps_pool.tile([P, 512], f32, tag="ps")
            for _ in range(110):
                nc.tensor.matmul(
                    warm_ps[:, :P], wtile[:], wtile[:], start=True, stop=True
                )

            xta_sb = xt_pool.tile([P, KO, c0_n], bf16)
            xtb_sb = (
                xt_pool.tile([P, KO, C - c0_n], bf16, name="xtb_sb")
                if nch > 1
                else None
            )
            act_sb = act_pool.tile([P, MJ, C], bf16)
            # ffn1 chunk descriptors: (xt tile, local col, global col, n)
            xchunks = [(xta_sb, 0, c0_0, c0_n)] + [
                (xtb_sb, c0 - c0_n, c0, cn) for c0, cn in chunks[1:]
            ]
            w1_tiles = {}

            def issue_w1(m, ring, halves=False):
                t = w1_pool.tile([P, KO, P], bf16, tag="w1s", name=f"w1_{m}")
                if halves:
                    ring.dma_start(t[:, : KO // 2], w1_d.ap()[:, m, : KO // 2])
                    ring.dma_start(t[:, KO // 2 :], w1_d.ap()[:, m, KO // 2 :])
                else:
                    ring.dma_start(t[:], w1_d.ap()[:, m])
                w1_tiles[m] = t

            # --- Head DMA schedule. xt chunk-0 k-quads split across both
            # rings, interleaved with half-tile weight strips for the first
            # three j's, so pass A's strip 0 streams without stalls.
            def xtc0_quad(ring, k0):
                ring.dma_start(
                    xta_sb[:, k0 : k0 + 4], xta_d.ap()[:, k0 : k0 + 4]
                )

            issue_w1(0, nc.scalar, halves=True)
            xtc0_quad(nc.sync, 0)
            xtc0_quad(nc.scalar, 4)
            xtc0_quad(nc.sync, 8)
            xtc0_quad(nc.scalar, 12)
            issue_w1(1, nc.sync, halves=True)
            issue_w1(2, nc.scalar, halves=True)
            issue_w1(3, nc.sync, halves=True)
            issue_w1(4, nc.scalar)
            issue_w1(5, nc.sync)
            # Remaining strip pairs for block 0, then the rest of xt (needed
            # only by pass B, ~40us later), so weight pairs stay ahead.
            for m in range(6, 2 * JB):
                issue_w1(m, nc.sync if m % 2 else nc.scalar)
            if nch > 1:
                # Scheduling floor: xtb is needed only by pass B (~45us in);
                # without this the scheduler interleaves these bulky loads
                # ahead of the pass-A weight strips and starves the PE.
                with tc.tile_wait_until(0.018):
                    for si, k0 in enumerate(range(0, KO, 2)):
                        ring = nc.sync if si % 2 else nc.scalar
                        ring.dma_start(
                            xtb_sb[:, k0 : k0 + 2], xtb_d.ap()[:, k0 : k0 + 2]
                        )

            def g_pass(wg, j, cs):
                pg = [
                    ps_pool.tile([P, 512], f32, tag="ps", name=f"pg{j}_{ci}")
                    for ci in range(len(cs))
                ]
                for ko in range(KO):
                    for ci, (xs, lc0, c0, cn) in enumerate(cs):
                        nc.tensor.matmul(
                            pg[ci][:, :cn],
                            wg[:, ko],
                            xs[:, ko, lc0 : lc0 + cn],
                            start=(ko == 0),
                            stop=(ko == KO - 1),
                        )
                tg = []
                for ci, (xs, lc0, c0, cn) in enumerate(cs):
                    t = tg_pool.tile([P, 512], f32, tag="tg", name=f"tg{j}_{ci}")
                    nc.scalar.activation(t[:, :cn], pg[ci][:, :cn], act_fn)
                    tg.append(t)
                return tg

            def u_pass(wu, j, cs, tg):
                pu = [
                    ps_pool.tile([P, 512], f32, tag="ps", name=f"pu{j}_{ci}")
                    for ci in range(len(cs))
                ]
                for ko in range(KO):
                    for ci, (xs, lc0, c0, cn) in enumerate(cs):
                        nc.tensor.matmul(
                            pu[ci][:, :cn],
                            wu[:, ko],
                            xs[:, ko, lc0 : lc0 + cn],
                            start=(ko == 0),
                            stop=(ko == KO - 1),
                        )
                for ci, (xs, lc0, c0, cn) in enumerate(cs):
                    nc.vector.tensor_mul(
                        out=act_sb[:, j, c0 : c0 + cn],
                        in0=tg[ci][:, :cn],
                        in1=pu[ci][:, :cn],
                    )

            with nc.named_scope("ffn1"):
                for jb in range(0, MJ, JB):
                    if jb > 0:  # prefetch this block's strip pairs
                        for m in range(2 * jb, 2 * (jb + JB)):
                            issue_w1(m, nc.sync if m % 2 else nc.scalar)
                    # pass A: first chunk only
                    for j in range(jb, jb + JB):
                        tg = g_pass(w1_tiles[2 * j], j, xchunks[:1])
                        u_pass(w1_tiles[2 * j + 1], j, xchunks[:1], tg)
                    # pass B: remaining chunks on the resident weight tiles
                    if nch > 1:
                        for j in range(jb, jb + JB):
                            wg = w1_tiles.pop(2 * j)
                            wu = w1_tiles.pop(2 * j + 1)
                            tg = g_pass(wg, j, xchunks[1:])
                            u_pass(wu, j, xchunks[1:], tg)
                    else:
                        for j in range(jb, jb + JB):
                            w1_tiles.pop(2 * j)
                            w1_tiles.pop(2 * j + 1)

            w2_tiles = {}

            def issue_w2(i, ring):
                t = w2_pool.tile([P, MJ, P], bf16, tag="w2s", name=f"w2_{i}")
                ring.dma_start(t[:], w2_d.ap()[:, i])
                w2_tiles[i] = t

            for i in range(3):
                issue_w2(i, nc.sync if i % 2 else nc.scalar)
            with nc.named_scope("ffn2"):
                for i in range(KO):
                    if i + 3 < KO:
                        issue_w2(i + 3, nc.sync if i % 2 else nc.scalar)
                    w2t = w2_tiles.pop(i)
                    last = i == KO - 1
                    cs = list(chunks)
                    if last and cs[-1][1] > 192:
                        # Sub-split the final chunk and run the last tile
                        # chunk-inner so each chunk's drain overlaps the next
                        # chunk's chains; the tail is one small copy + DMA.
                        lc0, lcn = cs.pop()
                        h = lcn - 96
                        cs += [(lc0, h), (lc0 + h, 96)]

                    def drain(ci, c0, cn):
                        yo = yo_pool.tile(
                            [P, 512], bf16, tag="yo", name=f"yo{i}_{ci}"
                        )
                        if last and ci == len(cs) - 1:
                            nc.scalar.activation(yo[:, :cn], py[ci][:, :cn], ident)
                            ring = nc.scalar
                        elif last and ci == len(cs) - 2:
                            nc.vector.tensor_copy(out=yo[:, :cn], in_=py[ci][:, :cn])
                            ring = nc.sync
                        else:
                            nc.vector.tensor_copy(out=yo[:, :cn], in_=py[ci][:, :cn])
                            ring = nc.sync if (i + ci) % 2 else nc.scalar
                        ring.dma_start(yt_d.ap()[:, i, c0 : c0 + cn], yo[:, :cn])

                    if last:
                        py = []
                        for ci, (c0, cn) in enumerate(cs):
                            py.append(ps_pool.tile(
                                [P, 512], f32, tag="ps", name=f"py{i}_{ci}"
                            ))
                            for fo in range(MJ):
                                nc.tensor.matmul(
                                    py[ci][:, :cn],
                                    w2t[:, fo],
                                    act_sb[:, fo, c0 : c0 + cn],
                                    start=(fo == 0),
                                    stop=(fo == MJ - 1),
                                )
                            drain(ci, c0, cn)
                    else:
                        py = [
                            ps_pool.tile(
                                [P, 512], f32, tag="ps", name=f"py{i}_{ci}"
                            )
                            for ci in range(len(cs))
                        ]
                        for fo in range(MJ):
                            for ci, (c0, cn) in enumerate(cs):
                                nc.tensor.matmul(
                                    py[ci][:, :cn],
                                    w2t[:, fo],
                                    act_sb[:, fo, c0 : c0 + cn],
                                    start=(fo == 0),
                                    stop=(fo == MJ - 1),
                                )
                        for ci, (c0, cn) in enumerate(cs):
                            drain(ci, c0, cn)

    nc.compile()
    _BUILD_CACHE[key] = nc
    return nc


def _router(x, router_scale, gate_w):
    """Replicate the reference router ops exactly (same jax ops, default
    backend) so the top-2 expert selection bit-matches the reference."""
    import jax
    import jax.numpy as jnp

    x = jnp.asarray(x)
    router_scale = jnp.asarray(router_scale)
    gate_w = jnp.asarray(gate_w)
    _B, _L, d = x.shape
    h = x * jax.lax.rsqrt(jnp.mean(x * x, axis=-1, keepdims=True) + EPS)
    h = h * (d**-0.5) * router_scale
    logits = (h @ gate_w).astype(jnp.float32)
    probs = jax.nn.softmax(logits, axis=-1)
    w, idx = jax.lax.top_k(probs, TOP_K)
    w = w / jnp.clip(jnp.sum(w, axis=-1, keepdims=True), 1e-12)
    w = w.astype(x.dtype)
    return (
        np.asarray(idx).reshape(-1, TOP_K),
        np.asarray(w).reshape(-1, TOP_K).astype(np.float32),
    )


def _bf16_dtype():
    import concourse.mybir as mybir

    return mybir.dt.np(mybir.dt.bfloat16)


def _pack_w1(gate_up_e: np.ndarray) -> np.ndarray:
    """[D, 2F] -> [P, 2*MJ, KO, P] bf16, gate/up 128-col strips interleaved."""
    g = gate_up_e[:, :F].reshape(D, MJ, P)
    u = gate_up_e[:, F:].reshape(D, MJ, P)
    w1p = np.empty((D, 2 * MJ, P), np.float32)
    w1p[:, 0::2] = g
    w1p[:, 1::2] = u
    # [D, 2MJ, P] -> [KO, P, 2MJ, P] -> [P, 2MJ, KO, P]
    return np.ascontiguousarray(
        w1p.reshape(KO, P, 2 * MJ, P).transpose(1, 2, 0, 3)
    ).astype(_bf16_dtype())


def _pack_w2(down_e: np.ndarray) -> np.ndarray:
    """[F, D] -> [P, KO, MJ, P] bf16 (w2[p, i, fo, q] = W2[fo*128+p, i*128+q])."""
    return np.ascontiguousarray(
        down_e.reshape(MJ, P, KO, P).transpose(1, 2, 0, 3)
    ).astype(_bf16_dtype())


def run_moe(x, router_scale, gate_w, gate_up, down, per_expert_scale, trace=False):
    from concourse import bass_utils

    x = np.asarray(x, dtype=np.float32)
    router_scale = np.asarray(router_scale, dtype=np.float32)
    gate_w = np.asarray(gate_w, dtype=np.float32)
    gate_up = np.asarray(gate_up, dtype=np.float32)
    down = np.asarray(down, dtype=np.float32)
    per_expert_scale = np.asarray(per_expert_scale, dtype=np.float32)

    B, L, d = x.shape
    N = B * L
    assert d == D and gate_up.shape == (E, D, 2 * F) and down.shape == (E, F, D)

    idxf, wf = _router(x, router_scale, gate_w)

    pair_expert = idxf.reshape(-1)
    pair_token = np.repeat(np.arange(N), TOP_K)
    pair_w = wf.reshape(-1) * per_expert_scale[pair_expert]

    order = np.argsort(pair_expert, kind="stable")
    tok_o = pair_token[order]
    w_o = pair_w[order]
    counts = np.bincount(pair_expert, minlength=E)
    offs = np.zeros(E + 1, np.int64)
    offs[1:] = np.cumsum(counts)

    # SBUF budget caps per-launch capacity; extreme routing imbalance falls
    # back to multiple launches over row segments of each expert's list.
    CMAX = 1296
    nseg = max(1, -(-int(counts.max()) // CMAX))
    seg_cap = -(-int(counts.max()) // nseg)
    C = max(64, -(-seg_cap // 4) * 4)

    nc = _build(C)
    c0_n = _chunks_of(C)[0][1]

    bf16 = _bf16_dtype()
    xf = x.reshape(N, D)
    w1_packed = [_pack_w1(gate_up[e]) for e in range(E)]
    w2_packed = [_pack_w2(down[e]) for e in range(E)]

    contrib = np.empty((len(tok_o), D), np.float32)
    res = None
    for s in range(nseg):
        in_maps = []
        ranges = []
        for e in range(E):
            lo = min(offs[e] + s * C, offs[e + 1])
            hi = min(lo + C, offs[e + 1])
            toks = tok_o[lo:hi]
            ranges.append((lo, hi))
            xg = np.zeros((C, D), np.float32)
            xg[: len(toks)] = xf[toks]
            xt = xg.T.reshape(KO, P, C).transpose(1, 0, 2).astype(bf16)
            im = {
                "xta": np.ascontiguousarray(xt[:, :, :c0_n]),
                "w1": w1_packed[e],
                "w2": w2_packed[e],
            }
            if C > c0_n:
                im["xtb"] = np.ascontiguousarray(xt[:, :, c0_n:])
            in_maps.append(im)

        res = bass_utils.run_bass_kernel_spmd(
            nc, in_maps, core_ids=list(range(E)), trace=trace and s == 0
        )
        for e in range(E):
            lo, hi = ranges[e]
            yt = np.asarray(res.results[e]["yt"]).astype(np.float32)
            ytd = yt.transpose(1, 0, 2).reshape(D, C)  # [D, C]
            contrib[lo:hi] = ytd[:, : hi - lo].T

    contrib *= w_o[:, None]

    s = np.argsort(tok_o, kind="stable")
    tok_s = tok_o[s]
    out = np.zeros((N, D), np.float32)
    if len(tok_s) == 2 * N and np.array_equal(tok_s[0::2], tok_s[1::2]):
        cs = contrib[s]
        out[tok_s[0::2]] = cs[0::2] + cs[1::2]
    else:  # defensive fallback (duplicate experts per token can't happen)
        np.add.at(out, tok_o, contrib)
    return out.reshape(B, L, D), res


def kernel(x, router_scale, gate_w, gate_up, down, per_expert_scale):
    out, _ = run_moe(x, router_scale, gate_w, gate_up, down, per_expert_scale)
    return out


# revision 22
# speedup vs baseline: 1.0125x; 1.0002x over previous
"""MoE layer (top-2 of 8 experts) on 8 TRN2 NeuronCores, expert-parallel.

Host side: router (exact replica of the reference jax ops, so top-k
selection bit-matches), token gather by expert assignment, weight
repacking into DMA-friendly bf16 layouts, and the final weighted
scatter-add.

Device side (one expert per core, SPMD): the full expert FFN
    h = X @ W1 ; act = gelu(h_gate) * h_up ; Y = act @ W2
in bf16 operands with fp32 PSUM accumulation (~4e-3 rel err), all
activations kept transposed (tokens on the free axis).

Schedule notes:
  * bf16 halves all DMA traffic and SBUF footprint vs fp32.
  * PE warm-up matmuls on a zeroed tile run during the initial DMA fill
    so the HAM clock gate releases to 8/8 (2.4 GHz) before real work.
  * ffn1 runs in j-blocks of 8 strips, two passes per block: pass A
    processes only the first token chunk (small xt footprint -> the
    first strip is compute-ready ~1us after the DMA rings open), pass B
    processes the remaining chunks on the same still-resident weight
    tiles. xt streams k-major, split across both HWDGE rings.
  * ko-outer matmul order reuses each stationary weight tile across all
    chunks of a pass (the bf16 LDWEIGHTS hides under the matmuls).
  * w2 streams during ffn2; output DMAs alternate rings; the final
    chunk is sub-split with its drain fanned across scalar+vector and
    both rings to shorten the kernel tail.

Self-contained: only library imports (numpy/jax/concourse), no file reads.
"""

import numpy as np

TOP_K = 2
EPS = 1e-6
P = 128
D = 2048
F = 2048  # expert hidden dim (ED)
E = 8
KO = D // P  # 16 K-tiles for matmul1 / output D-tiles
MJ = F // P  # 16 gate/up tile pairs; also K-tiles for matmul2
JB = 8  # ffn1 strip-block size (weights stay resident across both passes)

_BUILD_CACHE: dict = {}

# Activation for the gate branch. CoreSim doesn't implement Gelu, so tests
# can set this to "Identity" for structural sim validation.
ACT_FN = "Gelu"


def _chunks_of(C: int) -> list[tuple[int, int]]:
    """Split the token-capacity free axis into matmul chunks <= 512.

    512 fp32 PSUM values = exactly one 2 KiB bank, so each chunk's
    accumulator stays bank-aligned.
    """
    if C <= 512:
        return [(0, C)]
    nch = -(-C // 512)
    base = C // nch
    base -= base % 4
    sizes = [base] * nch
    rem = C - base * nch
    i = 0
    while rem > 0:
        add = min(4, rem)
        sizes[i % nch] += add
        rem -= add
        i += 1
    out = []
    off = 0
    for s in sizes:
        out.append((off, s))
        off += s
    assert off == C
    return out


def _build(C: int):
    """Build + compile the per-core expert-FFN bass program for capacity C."""
    key = (C, ACT_FN)
    if key in _BUILD_CACHE:
        return _BUILD_CACHE[key]

    import concourse.bacc as bacc
    import concourse.mybir as mybir
    import concourse.tile as tile
    f32 = mybir.dt.float32
    bf16 = mybir.dt.bfloat16
    act_fn = getattr(mybir.ActivationFunctionType, ACT_FN)
    ident = mybir.ActivationFunctionType.Identity
    chunks = _chunks_of(C)
    nch = len(chunks)
    assert nch <= 4

    nc = bacc.Bacc(
        "TRN2", target_bir_lowering=False, debug=False, enable_asserts=False
    )
    # Packed layouts (host pre-transposed, partition-major):
    #   xt[p, ko, c]    = X^T[ko*128+p, c]          (tokens on free axis)
    #   w1[p, m, ko, q] = W1perm[ko*128+p, m*128+q] (m: g0,u0,g1,u1,... strips)
    #   w2[p, i, fo, q] = W2[fo*128+p, i*128+q]
    #   yt[p, io, c]    = Y^T[io*128+p, c]
    c0_0, c0_n = chunks[0]
    # xt ships as two tensors so the chunk-0 head DMAs read contiguous
    # >=2KB per-partition lines (full DMA efficiency) before the rest lands.
    xta_d = nc.dram_tensor("xta", [P, KO, c0_n], bf16, kind="ExternalInput")
    xtb_d = (
        nc.dram_tensor("xtb", [P, KO, C - c0_n], bf16, kind="ExternalInput")
        if nch > 1
        else None
    )
    w1_d = nc.dram_tensor("w1", [P, 2 * MJ, KO, P], bf16, kind="ExternalInput")
    w2_d = nc.dram_tensor("w2", [P, KO, MJ, P], bf16, kind="ExternalInput")
    yt_d = nc.dram_tensor("yt", [P, KO, C], bf16, kind="ExternalOutput")

    with tile.TileContext(nc) as tc:
        with (
            tc.tile_pool(name="xt", bufs=1) as xt_pool,
            tc.tile_pool(name="act", bufs=1) as act_pool,
            tc.tile_pool(name="w1", bufs=2 * (JB + 2)) as w1_pool,
            tc.tile_pool(name="w2", bufs=5) as w2_pool,
            tc.tile_pool(name="tg", bufs=6) as tg_pool,
            tc.tile_pool(name="yo", bufs=6) as yo_pool,
            tc.tile_pool(name="wm", bufs=1) as wm_pool,
            tc.tile_pool(name="ps", bufs=8, space="PSUM") as ps_pool,
        ):
            # --- PE warm-up: dummy matmuls on a zeroed tile keep the HAM
            # clock-gate activity window busy while the first DMAs land.
            wtile = wm_pool.tile([P, P], bf16)
            nc.gpsimd.memset(wtile[:], 0.0)
            warm_ps = ps_pool.tile([P, 512], f32, tag="ps")
            for _ in range(88):
                nc.tensor.matmul(
                    warm_ps[:, :P], wtile[:], wtile[:], start=True, stop=True
                )

            xta_sb = xt_pool.tile([P, KO, c0_n], bf16)
            xtb_sb = (
                xt_pool.tile([P, KO, C - c0_n], bf16, name="xtb_sb")
                if nch > 1
                else None
            )
            act_sb = act_pool.tile([P, MJ, C], bf16)
            # ffn1 chunk descriptors: (xt tile, local col, global col, n)
            xchunks = [(xta_sb, 0, c0_0, c0_n)] + [
                (xtb_sb, c0 - c0_n, c0, cn) for c0, cn in chunks[1:]
            ]
            w1_tiles = {}

            def issue_w1(m, ring, halves=False):
                t = w1_pool.tile([P, KO, P], bf16, tag="w1s", name=f"w1_{m}")
                if halves:
                    ring.dma_start(t[:, : KO // 2], w1_d.ap()[:, m, : KO // 2])
                    ring.dma_start(t[:, KO // 2 :], w1_d.ap()[:, m, KO // 2 :])
                else:
                    ring.dma_start(t[:], w1_d.ap()[:, m])
                w1_tiles[m] = t

            # --- Head DMA schedule. xt chunk-0 k-quads split across both
            # rings, interleaved with half-tile weight strips for the first
            # three j's, so pass A's strip 0 streams without stalls.
            def xtc0_quad(ring, k0):
                ring.dma_start(
                    xta_sb[:, k0 : k0 + 4], xta_d.ap()[:, k0 : k0 + 4]
                )

            issue_w1(0, nc.scalar, halves=True)
            xtc0_quad(nc.sync, 0)
            xtc0_quad(nc.scalar, 4)
            xtc0_quad(nc.sync, 8)
            xtc0_quad(nc.scalar, 12)
            issue_w1(1, nc.sync, halves=True)
            issue_w1(2, nc.scalar, halves=True)
            issue_w1(3, nc.sync, halves=True)
            issue_w1(4, nc.scalar)
            issue_w1(5, nc.sync)
            # Remaining strip pairs for block 0, then the rest of xt (needed
            # only by pass B, ~40us later), so weight pairs stay ahead.
            for m in range(6, 2 * JB):
                issue_w1(m, nc.sync if m % 2 else nc.scalar)
            if nch > 1:
                # Scheduling floor: xtb is needed only by pass B (~45us in);
                # without this the scheduler interleaves these bulky loads
                # ahead of the pass-A weight strips and starves the PE.
                with tc.tile_wait_until(0.018):
                    for si, k0 in enumerate(range(0, KO, 2)):
                        ring = nc.sync if si % 2 else nc.scalar
                        ring.dma_start(
                            xtb_sb[:, k0 : k0 + 2], xtb_d.ap()[:, k0 : k0 + 2]
                        )

            def g_pass(wg, j, cs):
                pg = [
                    ps_pool.tile([P, 512], f32, tag="ps", name=f"pg{j}_{ci}")
                    for ci in range(len(cs))
                ]
                for ko in range(KO):
                    for ci, (xs, lc0, c0, cn) in enumerate(cs):
                        nc.tensor.matmul(
                            pg[ci][:, :cn],
                            wg[:, ko],
                            xs[:, ko, lc0 : lc0 + cn],
                            start=(ko == 0),
                            stop=(ko == KO - 1),
                        )
                tg = []
                for ci, (xs, lc0, c0, cn) in enumerate(cs):
                    t = tg_pool.tile([P, 512], f32, tag="tg", name=f"tg{j}_{ci}")
                    nc.scalar.activation(t[:, :cn], pg[ci][:, :cn], act_fn)
                    tg.append(t)
                return tg

            def u_pass(wu, j, cs, tg):
                pu = [
                    ps_pool.tile([P, 512], f32, tag="ps", name=f"pu{j}_{ci}")
                    for ci in range(len(cs))
                ]
                for ko in range(KO):
                    for ci, (xs, lc0, c0, cn) in enumerate(cs):
                        nc.tensor.matmul(
                            pu[ci][:, :cn],
                            wu[:, ko],
                            xs[:, ko, lc0 : lc0 + cn],
                            start=(ko == 0),
                            stop=(ko == KO - 1),
                        )
                for ci, (xs, lc0, c0, cn) in enumerate(cs):
                    nc.vector.tensor_mul(
                        out=act_sb[:, j, c0 : c0 + cn],
                        in0=tg[ci][:, :cn],
                        in1=pu[ci][:, :cn],
                    )

            with nc.named_scope("ffn1"):
                for jb in range(0, MJ, JB):
                    if jb > 0:  # prefetch this block's strip pairs
                        for m in range(2 * jb, 2 * (jb + JB)):
                            issue_w1(m, nc.sync if m % 2 else nc.scalar)
                    # pass A: first chunk only
                    for j in range(jb, jb + JB):
                        tg = g_pass(w1_tiles[2 * j], j, xchunks[:1])
                        u_pass(w1_tiles[2 * j + 1], j, xchunks[:1], tg)
                    # pass B: remaining chunks on the resident weight tiles
                    if nch > 1:
                        for j in range(jb, jb + JB):
                            wg = w1_tiles.pop(2 * j)
                            wu = w1_tiles.pop(2 * j + 1)
                            tg = g_pass(wg, j, xchunks[1:])
                            u_pass(wu, j, xchunks[1:], tg)
                    else:
                        for j in range(jb, jb + JB):
                            w1_tiles.pop(2 * j)
                            w1_tiles.pop(2 * j + 1)

            w2_tiles = {}

            def issue_w2(i, ring):
                t = w2_pool.tile([P, MJ, P], bf16, tag="w2s", name=f"w2_{i}")
                ring.dma_start(t[:], w2_d.ap()[:, i])
                w2_tiles[i] = t

            for i in range(3):
                issue_w2(i, nc.sync if i % 2 else nc.scalar)
            with nc.named_scope("ffn2"):
                for i in range(KO):
                    if i + 3 < KO:
                        issue_w2(i + 3, nc.sync if i % 2 else nc.scalar)
                    w2t = w2_tiles.pop(i)
                    last = i == KO - 1
                    cs = list(chunks)
                    if last and cs[-1][1] > 192:
                        # Sub-split the final chunk and run the last tile
                        # chunk-inner so each chunk's drain overlaps the next
                        # chunk's chains; the tail is one small copy + DMA.
                        lc0, lcn = cs.pop()
                        h = lcn - 96
                        cs += [(lc0, h), (lc0 + h, 96)]

                    def drain(ci, c0, cn):
                        yo = yo_pool.tile(
                            [P, 512], bf16, tag="yo", name=f"yo{i}_{ci}"
                        )
                        if last and ci == len(cs) - 1:
                            nc.scalar.activation(yo[:, :cn], py[ci][:, :cn], ident)
                            ring = nc.scalar
                        elif last and ci == len(cs) - 2:
                            nc.vector.tensor_copy(out=yo[:, :cn], in_=py[ci][:, :cn])
                            ring = nc.sync
                        else:
                            nc.vector.tensor_copy(out=yo[:, :cn], in_=py[ci][:, :cn])
                            ring = nc.sync if (i + ci) % 2 else nc.scalar
                        ring.dma_start(yt_d.ap()[:, i, c0 : c0 + cn], yo[:, :cn])

                    if last:
                        py = []
                        for ci, (c0, cn) in enumerate(cs):
                            py.append(ps_pool.tile(
                                [P, 512], f32, tag="ps", name=f"py{i}_{ci}"
                            ))
                            for fo in range(MJ):
                                nc.tensor.matmul(
                                    py[ci][:, :cn],
                                    w2t[:, fo],
                                    act_sb[:, fo, c0 : c0 + cn],
                                    start=(fo == 0),
                                    stop=(fo == MJ - 1),
                                )
                            drain(ci, c0, cn)
                    else:
                        py = [
                            ps_pool.tile(
                                [P, 512], f32, tag="ps", name=f"py{i}_{ci}"
                            )
                            for ci in range(len(cs))
                        ]
                        for fo in range(MJ):
                            for ci, (c0, cn) in enumerate(cs):
                                nc.tensor.matmul(
                                    py[ci][:, :cn],
                                    w2t[:, fo],
                                    act_sb[:, fo, c0 : c0 + cn],
                                    start=(fo == 0),
                                    stop=(fo == MJ - 1),
                                )
                        for ci, (c0, cn) in enumerate(cs):
                            drain(ci, c0, cn)

    nc.compile()
    _BUILD_CACHE[key] = nc
    return nc


def _router(x, router_scale, gate_w):
    """Replicate the reference router ops exactly (same jax ops, default
    backend) so the top-2 expert selection bit-matches the reference."""
    import jax
    import jax.numpy as jnp

    x = jnp.asarray(x)
    router_scale = jnp.asarray(router_scale)
    gate_w = jnp.asarray(gate_w)
    _B, _L, d = x.shape
    h = x * jax.lax.rsqrt(jnp.mean(x * x, axis=-1, keepdims=True) + EPS)
    h = h * (d**-0.5) * router_scale
    logits = (h @ gate_w).astype(jnp.float32)
    probs = jax.nn.softmax(logits, axis=-1)
    w, idx = jax.lax.top_k(probs, TOP_K)
    w = w / jnp.clip(jnp.sum(w, axis=-1, keepdims=True), 1e-12)
    w = w.astype(x.dtype)
    return (
        np.asarray(idx).reshape(-1, TOP_K),
        np.asarray(w).reshape(-1, TOP_K).astype(np.float32),
    )


def _bf16_dtype():
    import concourse.mybir as mybir

    return mybir.dt.np(mybir.dt.bfloat16)


def _pack_w1(gate_up_e: np.ndarray) -> np.ndarray:
    """[D, 2F] -> [P, 2*MJ, KO, P] bf16, gate/up 128-col strips interleaved."""
    g = gate_up_e[:, :F].reshape(D, MJ, P)
    u = gate_up_e[:, F:].reshape(D, MJ, P)
    w1p = np.empty((D, 2 * MJ, P), np.float32)
    w1p[:, 0::2] = g
    w1p[:, 1::2] = u
    # [D, 2MJ, P] -> [KO, P, 2MJ, P] -> [P, 2MJ, KO, P]
    return np.ascontiguousarray(
        w1p.reshape(KO, P, 2 * MJ, P).transpose(1, 2, 0, 3)
    ).astype(_bf16_dtype())


def _pack_w2(down_e: np.ndarray) -> np.ndarray:
    """[F, D] -> [P, KO, MJ, P] bf16 (w2[p, i, fo, q] = W2[fo*128+p, i*128+q])."""
    return np.ascontiguousarray(
        down_e.reshape(MJ, P, KO, P).transpose(1, 2, 0, 3)
    ).astype(_bf16_dtype())


def run_moe(x, router_scale, gate_w, gate_up, down, per_expert_scale, trace=False):
    from concourse import bass_utils

    x = np.asarray(x, dtype=np.float32)
    router_scale = np.asarray(router_scale, dtype=np.float32)
    gate_w = np.asarray(gate_w, dtype=np.float32)
    gate_up = np.asarray(gate_up, dtype=np.float32)
    down = np.asarray(down, dtype=np.float32)
    per_expert_scale = np.asarray(per_expert_scale, dtype=np.float32)

    B, L, d = x.shape
    N = B * L
    assert d == D and gate_up.shape == (E, D, 2 * F) and down.shape == (E, F, D)

    idxf, wf = _router(x, router_scale, gate_w)

    pair_expert = idxf.reshape(-1)
    pair_token = np.repeat(np.arange(N), TOP_K)
    pair_w = wf.reshape(-1) * per_expert_scale[pair_expert]

    order = np.argsort(pair_expert, kind="stable")
    tok_o = pair_token[order]
    w_o = pair_w[order]
    counts = np.bincount(pair_expert, minlength=E)
    offs = np.zeros(E + 1, np.int64)
    offs[1:] = np.cumsum(counts)

    # SBUF budget caps per-launch capacity; extreme routing imbalance falls
    # back to multiple launches over row segments of each expert's list.
    CMAX = 1296
    nseg = max(1, -(-int(counts.max()) // CMAX))
    seg_cap = -(-int(counts.max()) // nseg)
    C = max(64, -(-seg_cap // 4) * 4)

    nc = _build(C)
    c0_n = _chunks_of(C)[0][1]

    bf16 = _bf16_dtype()
    xf = x.reshape(N, D)
    w1_packed = [_pack_w1(gate_up[e]) for e in range(E)]
    w2_packed = [_pack_w2(down[e]) for e in range(E)]

    contrib = np.empty((len(tok_o), D), np.float32)
    res = None
    for s in range(nseg):
        in_maps = []
        ranges = []
        for e in range(E):
            lo = min(offs[e] + s * C, offs[e + 1])
            hi = min(lo + C, offs[e + 1])
            toks = tok_o[lo:hi]
            ranges.append((lo, hi))
            xg = np.zeros((C, D), np.float32)
            xg[: len(toks)] = xf[toks]
            xt = xg.T.reshape(KO, P, C).transpose(1, 0, 2).astype(bf16)
            im = {
                "xta": np.ascontiguousarray(xt[:, :, :c0_n]),
                "w1": w1_packed[e],
                "w2": w2_packed[e],
            }
            if C > c0_n:
                im["xtb"] = np.ascontiguousarray(xt[:, :, c0_n:])
            in_maps.append(im)

        res = bass_utils.run_bass_kernel_spmd(
            nc, in_maps, core_ids=list(range(E)), trace=trace and s == 0
        )
        for e in range(E):
            lo, hi = ranges[e]
            yt = np.asarray(res.results[e]["yt"]).astype(np.float32)
            ytd = yt.transpose(1, 0, 2).reshape(D, C)  # [D, C]
            contrib[lo:hi] = ytd[:, : hi - lo].T

    contrib *= w_o[:, None]

    s = np.argsort(tok_o, kind="stable")
    tok_s = tok_o[s]
    out = np.zeros((N, D), np.float32)
    if len(tok_s) == 2 * N and np.array_equal(tok_s[0::2], tok_s[1::2]):
        cs = contrib[s]
        out[tok_s[0::2]] = cs[0::2] + cs[1::2]
    else:  # defensive fallback (duplicate experts per token can't happen)
        np.add.at(out, tok_o, contrib)
    return out.reshape(B, L, D), res


def kernel(x, router_scale, gate_w, gate_up, down, per_expert_scale):
    out, _ = run_moe(x, router_scale, gate_w, gate_up, down, per_expert_scale)
    return out
